# revision 41
# baseline (speedup 1.0000x reference)
"""Trainium2 Bass kernel for causal top-8 sparse attention (nn_DGN7).

Math (see reference):
  A    = top-8 strictly-causal neighbours of each row by x.x^T similarity
  attn = softmax over the selected scores, score = (x Wq^T)(x Wk^T)^T/sqrt(32)
  out  = gelu_exact((mix*x + (1-mix)*attn@x) * gain + bias) * (softplus+0.01)

Sharding: 8 cores; core i handles batch i//4 and, for every prefix level
l=1..8, the 128-row tile g = 4*(l-1) + (i%4).  Every core runs an identical
static program over strips of width 512*l (l=1..8); total causal area is
exactly balanced across cores.

Numerics:
  - similarity strip kept in units of 2048*x.x' (selection is scale
    invariant).  Main term (32h).(64h)' in fp16 (exact power-of-2 scalings
    of h=fp16(x)); hi/lo cross terms h.l' + l.h' (l = fp16((x-h)*2048)) via
    ONE fp8e4m3 DoubleRow matmul per 128-chunk (2x PE rate).
  - top-8 via DVE Max8; per-row 8th value v8 defines the selection:
    mask = (sim < v8) * (-57344) built as an fp8e5m2 tile and ADDED to the
    scores through an identity matmul on the PE (keeps DVE off the
    score->exp->msg critical chain).  Near-tie rows are detected by a fused
    count pass (# of sim >= v8-20, via tensor_scalar accum_out) and
    recomputed exactly on the host (~70 of 8192 rows).
  - q/k/score/msg matmuls in fp16; softmax without max-shift (exp biased by
    -4); Z via ACT accum_out; normalisation (1/Z) on DVE after the msg
    matmul.  Device exports the normalised message in fp16; the host
    applies blend+gelu+scale exactly (float64 erf), so the device runs a
    single ACT function (Exp) with zero activation-table swaps.
  - gain/mix folded host-side into the msg operand: xbh = x*(1-mix)*gain.
Host does layout prep (transposes/fp16/fp8 piece casts), the final
blend/gelu/scale, the degenerate t=0 rows, and the flagged near-tie rows.
"""
import math
import numpy as np
import ml_dtypes

import concourse.bass as bass
import concourse.mybir as mybir
from concourse import bacc
from concourse.tile import TileContext
from concourse.bass_utils import run_bass_kernel_spmd

B, T, D = 2, 4096, 1024
DH = 32
P = 128
PANEL = 512
NLEV = 8
NPAN = 8
NCHUNK = D // P          # 8
NCORES = 8
FMIN = float(np.finfo(np.float32).min)
MASKVAL = -3.0e38        # sim diagonal mask (finite in bf16)
SCOREMASK = -57344.0     # score mask, exactly representable in fp8e5m2
SPLIT = 2048.0           # 2^11 lo-piece scale
ESHIFT = -4.0            # exp input bias (fp16 range safety)
GAPTHR = 20.0            # flag threshold, strip units (2048 * x.x')

f32 = mybir.dt.float32
f16 = mybir.dt.float16
bf16 = mybir.dt.bfloat16
f8 = mybir.dt.float8e4
f8e5 = mybir.dt.float8e5
DR = mybir.MatmulPerfMode.DoubleRow
FP8 = ml_dtypes.float8_e4m3
FP8E5 = ml_dtypes.float8_e5m2

_prog_cache = {}


def _build_program():
    nc = bacc.Bacc(trn_type="TRN2")

    # ---------------- DRAM I/O ----------------
    d_pan16 = nc.dram_tensor("pan16", [NPAN, P, NCHUNK, PANEL], f16,
                             kind="ExternalInput")      # (64h)^T panels
    d_pan8 = nc.dram_tensor("pan8", [NPAN, P, NCHUNK, 2, PANEL], f8,
                            kind="ExternalInput")       # (l8,h8)^T slabs
    d_xr16 = nc.dram_tensor("xr16", [NLEV, P, NCHUNK, P], f16,
                            kind="ExternalInput")       # (32h)^T own rows
    d_xr8 = nc.dram_tensor("xr8", [NLEV, P, NCHUNK, 2, P], f8,
                           kind="ExternalInput")        # (h8,l8)^T own rows
    d_xbh = nc.dram_tensor("xbh", [NPAN, P, 4, D], f16, kind="ExternalInput")
    d_wq = nc.dram_tensor("wq", [P, NCHUNK, DH], f16, kind="ExternalInput")
    d_wk = nc.dram_tensor("wk", [P, NCHUNK, DH], f16, kind="ExternalInput")
    d_maskdiag = nc.dram_tensor("maskdiag", [P, PANEL], bf16, kind="ExternalInput")
    d_ident16 = nc.dram_tensor("ident16", [P, P], f16, kind="ExternalInput")
    d_identbf = nc.dram_tensor("identbf", [P, P], bf16, kind="ExternalInput")
    d_identf8 = nc.dram_tensor("identf8", [P, P], f8e5, kind="ExternalInput")
    d_eshift = nc.dram_tensor("eshift", [P, 1], f32, kind="ExternalInput")
    d_out = nc.dram_tensor("out", [NLEV, P, D], f16, kind="ExternalOutput")
    d_cnt = nc.dram_tensor("cnt", [NLEV, P, 1], f32, kind="ExternalOutput")

    with TileContext(nc) as tc:
        with tc.tile_pool(name="const", bufs=1) as cpool, \
             tc.tile_pool(name="strips", bufs=1) as spool, \
             tc.tile_pool(name="big", bufs=1) as bpool, \
             tc.tile_pool(name="panels", bufs=2) as ppool, \
             tc.tile_pool(name="attn", bufs=1) as apool, \
             tc.tile_pool(name="masks", bufs=2) as kpool, \
             tc.tile_pool(name="msgx", bufs=3) as mpool, \
             tc.tile_pool(name="work", bufs=2) as wpool, \
             tc.tile_pool(name="work1", bufs=1) as w1pool, \
             tc.tile_pool(name="simP", bufs=2, space="PSUM") as simP, \
             tc.tile_pool(name="miscP", bufs=2, space="PSUM") as miscP, \
             tc.tile_pool(name="tranP", bufs=2, space="PSUM") as tranP, \
             tc.tile_pool(name="msgP", bufs=2, space="PSUM") as msgP:

            # ---------------- startup DMAs, critical-first ----------------
            # panel 0 split across the 3 DMA-capable queues, in the order
            # the iteration-0 sim tiles consume the pieces; consts follow
            pan16_0 = ppool.tile([P, NCHUNK, PANEL], f16, tag="pan16")
            pan8_0 = ppool.tile([P, NCHUNK, 2, PANEL], f8, tag="pan8")
            xr16_sb, xr8_sb = [], []
            for l in range(NLEV):
                t16 = bpool.tile([P, NCHUNK, P], f16, tag=f"xr16_{l}",
                                 name=f"xr16_{l}")
                t8 = bpool.tile([P, NCHUNK, 2, P], f8, tag=f"xr8_{l}",
                                name=f"xr8_{l}")
                xr16_sb.append(t16)
                xr8_sb.append(t8)
            nc.sync.dma_start(xr16_sb[0], d_xr16[0])
            nc.scalar.dma_start(pan16_0[:, 4:8], d_pan16[0][:, 4:8])
            nc.gpsimd.dma_start(pan8_0[:, 0:4], d_pan8[0][:, 0:4])
            nc.sync.dma_start(pan16_0[:, 0:4], d_pan16[0][:, 0:4])
            nc.scalar.dma_start(xr8_sb[0], d_xr8[0])
            nc.gpsimd.dma_start(pan8_0[:, 4:8], d_pan8[0][:, 4:8])
            # diag-mask + k/q weights are needed within the first ~20us
            identbf = cpool.tile([P, P], bf16)
            nc.scalar.dma_start(identbf, d_identbf.ap())
            maskdiag = cpool.tile([P, PANEL], bf16)
            nc.scalar.dma_start(maskdiag, d_maskdiag.ap())
            wq_sb = cpool.tile([P, NCHUNK, DH], f16)
            wk_sb = cpool.tile([P, NCHUNK, DH], f16)
            nc.scalar.dma_start(wk_sb, d_wk.ap())
            nc.scalar.dma_start(wq_sb, d_wq.ap())
            qrr = [nc.sync, nc.scalar, nc.gpsimd]
            for l in range(1, NLEV):
                qrr[(2 * l) % 3].dma_start(xr16_sb[l], d_xr16[l])
                qrr[(2 * l + 1) % 3].dma_start(xr8_sb[l], d_xr8[l])
            ident16 = cpool.tile([P, P], f16)
            nc.sync.dma_start(ident16, d_ident16.ap())
            identf8 = cpool.tile([P, P], f8e5)
            nc.sync.dma_start(identf8, d_identf8.ap())
            eshiftcol = cpool.tile([P, 1], f32)
            nc.gpsimd.dma_start(eshiftcol, d_eshift.ap())

            # PE warm-up: the HAM clock gate keeps an idle PE at 1.2 GHz and
            # only releases to 2.4 GHz after ~3.4us of sustained activity.
            # Burn the startup DMA wait on dummy matmuls so the first real
            # sim tiles run at full clock.
            warm = cpool.tile([P, P], f16)
            nc.vector.memset(warm, 0.25)
            wps = miscP.tile([P, PANEL], f32, tag="misc")
            for _ in range(150):
                nc.tensor.matmul(wps[:, :P], warm, warm, start=True, stop=True)

            kT_sb = cpool.tile([DH, T], f16)        # k^T, filled per panel
            strip = [spool.tile([P, PANEL * (l + 1)], f32, tag=f"strip{l}",
                                name=f"strip{l}")
                     for l in range(NLEV)]
            qT_sb = cpool.tile([DH, P], f16)
            attnT = apool.tile([P, 4 * NLEV, P], f16, tag="attnT")
            # resident msg panels 0/1 (read by 8/7 levels)
            xres = [cpool.tile([P, 4, D], f16, tag=f"xres{c}",
                               name=f"xres{c}") for c in range(2)]

            def load_panel(p):
                t16 = ppool.tile([P, NCHUNK, PANEL], f16, tag="pan16")
                t8 = ppool.tile([P, NCHUNK, 2, PANEL], f8, tag="pan8")
                nc.sync.dma_start(t16, d_pan16[p])
                nc.gpsimd.dma_start(t8, d_pan8[p])
                return t16, t8

            def load_xbh(c, qi):
                # halves on both rings: half the arrival latency
                xbh = mpool.tile([P, 4, D], f16, tag="xbh")
                qa, qb = (nc.gpsimd, nc.sync) if qi % 2 else (nc.sync, nc.gpsimd)
                qa.dma_start(xbh[:, 0:2], d_xbh[c][:, 0:2])
                qb.dma_start(xbh[:, 2:4], d_xbh[c][:, 2:4])
                return xbh

            def emit_sim_tile(l, p, p16, p8):
                ps = simP.tile([P, PANEL], f32, tag="sim")
                li = l - 1
                last = (p == l - 1)
                n = NCHUNK + NCHUNK + (1 if last else 0)
                i = 0
                for c in range(NCHUNK):
                    nc.tensor.matmul(ps, xr16_sb[li][:, c], p16[:, c],
                                     start=(i == 0), stop=(i == n - 1))
                    i += 1
                for c in range(NCHUNK):
                    nc.tensor.matmul(ps, xr8_sb[li][:, c], p8[:, c],
                                     start=False, stop=(i == n - 1),
                                     perf_mode=DR)
                    i += 1
                if last:
                    nc.tensor.matmul(ps, identbf, maskdiag,
                                     start=False, stop=True)
                nc.scalar.copy(strip[li][:, PANEL * p:PANEL * (p + 1)], ps)

            def issue_selection(l):
                """max8 -> v8; fused near-tie count; fp8e5 additive mask."""
                li = l - 1
                w = PANEL * l
                st = strip[li][:, :w]
                top8 = w1pool.tile([P, 8], f32, tag="top8")
                nc.vector.max(out=top8, in_=st)
                v8c = w1pool.tile([P, 1], f32, tag="v8")
                nc.vector.tensor_reduce(out=v8c, in_=top8,
                                        op=mybir.AluOpType.min,
                                        axis=mybir.AxisListType.X)
                v8t = w1pool.tile([P, 1], f32, tag="v8t")
                nc.vector.tensor_scalar_sub(v8t, v8c, GAPTHR)
                mask = kpool.tile([P, PANEL * NLEV], f8e5, tag="mask")
                cntc = w1pool.tile([P, 1], f32, tag=f"cnt{li % 2}")
                # count of sim >= v8 - GAPTHR (top-8 included; >8 => near-tie)
                nc.vector.tensor_scalar(mask[:, :w], st, v8t, scalar2=0.0,
                                        op0=mybir.AluOpType.is_ge,
                                        op1=mybir.AluOpType.add,
                                        accum_out=cntc)
                nc.scalar.dma_start(d_cnt[li], cntc)
                # additive score mask: 0 on top-8, -57344 elsewhere
                nc.vector.tensor_scalar(mask[:, :w], st, v8c,
                                        scalar2=SCOREMASK,
                                        op0=mybir.AluOpType.is_lt,
                                        op1=mybir.AluOpType.mult)
                return mask

            def emit_kT(p, p16):
                kps = miscP.tile([P, PANEL], f32, tag="misc")
                for c in range(NCHUNK):
                    nc.tensor.matmul(kps[:DH, :], wk_sb[:, c], p16[:, c],
                                     start=(c == 0), stop=(c == NCHUNK - 1))
                nc.scalar.copy(kT_sb[:, PANEL * p:PANEL * (p + 1)], kps[:DH, :])

            def level_compute_start(l, mask, xbhs):
                """Emit q^T plus the first two score/exp stages for level l;
                the returned finish() emits the rest.  The split lets the
                first exps (ACT) run underneath the iteration's sim tiles
                (PE), so the transpose/msg pipeline starts hot.
                xbhs: prefetched msg-panel tiles for c=2,3 (from the previous
                level); finish() returns the same dict for level l+1."""
                li = l - 1
                # --- q^T for this level ---
                qps = miscP.tile([P, PANEL], f32, tag="misc")
                for c in range(NCHUNK):
                    nc.tensor.matmul(qps[:DH, :P], wq_sb[:, c],
                                     xr16_sb[li][:, c],
                                     start=(c == 0), stop=(c == NCHUNK - 1))
                nc.scalar.copy(qT_sb, qps[:DH, :P])
                zcols = w1pool.tile([P, NLEV], f32, tag="zcols")
                mps = [msgP.tile([P, PANEL], f32, tag="msg", name=f"mp{k}")
                       for k in range(2)]
                nblk = 4 * l
                if l >= 5:
                    xbhs[4] = load_xbh(4, 0)

                def score_stage(c):
                    # no separate diag mask here: strip diag entries carry
                    # sim-3e38, so the v8-mask already outputs -57344 there
                    # (rows with v8=-3e38 are count-flagged and host-fixed)
                    sps = miscP.tile([P, PANEL], f32, tag="misc")
                    nc.tensor.matmul(sps, qT_sb,
                                     kT_sb[:, PANEL * c:PANEL * (c + 1)],
                                     start=True, stop=False)
                    nc.tensor.matmul(sps, identf8,
                                     mask[:, PANEL * c:PANEL * (c + 1)],
                                     start=False, stop=True)
                    au = wpool.tile([P, PANEL], f16, tag="au")
                    nc.scalar.activation(au, sps,
                                         mybir.ActivationFunctionType.Exp,
                                         bias=eshiftcol, scale=1.0,
                                         accum_out=zcols[:, c:c + 1])
                    return au

                def consume_stage(c, au):
                    tp = tranP.tile([P, PANEL], f16, tag="tran")
                    for q in range(4):
                        nc.tensor.matmul(tp[:, P * q:P * (q + 1)],
                                         au[:, P * q:P * (q + 1)], ident16,
                                         is_transpose=True,
                                         start=(q == 0), stop=(q == 3))
                    # final level: DVE is idle there, ACT is the chain
                    # bottleneck, so move the attn^T copies off ACT
                    cpeng = nc.vector.tensor_copy if l == NLEV else nc.scalar.copy
                    cpeng(
                        attnT[:, 4 * c:4 * (c + 1)].rearrange("p b t -> p (b t)"),
                        tp)
                    xbh = xres[c] if c < 2 else xbhs[c]
                    for k in range(2):
                        for sb in range(4):
                            blk = 4 * c + sb
                            nc.tensor.matmul(
                                mps[k], attnT[:, blk],
                                xbh[:, sb, PANEL * k:PANEL * (k + 1)],
                                start=(blk == 0), stop=(blk == nblk - 1))
                    # prefetch 3 panels ahead (after this panel's readers
                    # are emitted, so the buffer-reuse WAR is well-formed)
                    if c >= 2 and c + 3 < l:
                        xbhs[c + 3] = load_xbh(c + 3, c)

                nhead = min(2, l)
                aus = [score_stage(c) for c in range(nhead)]

                def finish():
                    # 2-deep software pipeline: consume runs nhead behind
                    for c in range(nhead, l):
                        consume_stage(c - nhead, aus[c - nhead])
                        aus.append(score_stage(c))
                    for c in range(l - nhead, l - 1):
                        consume_stage(c, aus[c])
                    # Z chain before the last consume: DVE computes 1/Z
                    # while the PE runs the final msg matmuls
                    zsum = w1pool.tile([P, 1], f32, tag="zsum")
                    nc.vector.tensor_reduce(
                        out=zsum, in_=zcols[:, :l], op=mybir.AluOpType.add,
                        axis=mybir.AxisListType.X)
                    nc.vector.tensor_scalar_max(zsum, zsum, 1e-30)
                    zrec = w1pool.tile([P, 1], f32, tag="zrec")
                    nc.vector.reciprocal(zrec, zsum)
                    consume_stage(l - 1, aus[l - 1])
                    # prefetch the next level's first JIT msg panels
                    nxt_xbhs = {}
                    if l < NLEV:
                        if l + 1 >= 3:
                            nxt_xbhs[2] = load_xbh(2, 0)
                        if l + 1 >= 4:
                            nxt_xbhs[3] = load_xbh(3, 1)
                    for k in range(2):
                        sl = slice(PANEL * k, PANEL * (k + 1))
                        gh = w1pool.tile([P, PANEL], f16, tag=f"g{k}",
                                         name=f"g{k}")
                        nc.vector.tensor_scalar_mul(gh, mps[k], zrec)
                        # outputs go on the scalar ring: compute-gated
                        # triggers must not head-of-line-block input rings
                        nc.scalar.dma_start(d_out[li][:, sl], gh)
                    return nxt_xbhs
                return finish

            # ---------------- main pipeline ----------------
            cur = (pan16_0, pan8_0)
            masks = {}
            pre = {}
            for p in range(NPAN):
                nxt = load_panel(p + 1) if p >= 1 and p + 1 < NPAN else None
                p16, p8 = cur
                # level p+1's last tile first: completes its strip so the
                # selection (DVE) overlaps the rest of this iteration
                emit_sim_tile(p + 1, p, p16, p8)
                masks[p + 1] = issue_selection(p + 1)
                fin = None
                if p >= 1:
                    # head of level p (qT + first scores/exps) before the
                    # sim tiles; p=0: wk lands late in the startup burst and
                    # nothing needs kT(0) before iteration 1 — emit it after
                    # the iteration-0 sim tiles instead
                    fin = level_compute_start(p, masks.pop(p), pre)
                    emit_kT(p, p16)
                if p == 0:
                    # panel 1 + first resident msg panel; xres[1] (first
                    # needed in iteration 2) waits so it doesn't compete
                    # with the startup burst for HBM bandwidth
                    nxt = load_panel(1)
                    nc.sync.dma_start(xres[0], d_xbh[0])
                if p == 1:
                    nc.gpsimd.dma_start(xres[1], d_xbh[1])
                # remaining sim tiles BEFORE level_compute: panel p's last
                # readers then sit early in the PE stream, so panel p+2's
                # buffer-reuse DMA (issued next iteration) is not WAR-gated
                # on the tail of this iteration
                for l in range(p + 2, NLEV + 1):
                    emit_sim_tile(l, p, p16, p8)
                if p == 0:
                    emit_kT(p, p16)
                if p >= 1:
                    pre = fin()
                cur = nxt
            level_compute_start(NLEV, masks.pop(NLEV), pre)()

    nc.compile()
    return nc


def _gelu_exact_np(v):
    er = np.array([math.erf(float(t) / math.sqrt(2.0)) for t in v.ravel()],
                  dtype=np.float64).reshape(v.shape)
    return v * 0.5 * (1.0 + er)


def _erf_vec(v):
    try:
        from scipy.special import erf
        return erf(v)
    except Exception:
        from jax.scipy.special import erf as jerf
        import jax
        with jax.default_device(jax.devices("cpu")[0]):
            return np.asarray(jerf(v))


def _fix_row(out, xb, W_q, W_k, gain, bias, mix, scale, t):
    """Recompute row t of batch xb exactly (host, fp32 selection/fp64 tail)."""
    kk = min(8, t)
    if kk == 0:
        return  # t=0 handled by caller
    srow = xb[:t] @ xb[t]                       # fp32 similarities (j < t)
    idx = np.argsort(-srow, kind="stable")[:kk]
    q = (xb[t:t + 1] @ W_q.T).astype(np.float64)[0] / math.sqrt(DH)
    kv = (xb[idx] @ W_k.T).astype(np.float64)
    sc = kv @ q
    sc -= sc.max()
    e = np.exp(sc)
    a = e / e.sum()
    msg = a @ xb[idx].astype(np.float64)
    blended = mix * xb[t].astype(np.float64) + (1.0 - mix) * msg
    pre = blended * gain.astype(np.float64) + bias.astype(np.float64)
    out[t] = (_gelu_exact_np(pre) * scale).astype(np.float32)


def kernel(x, W_q, W_k, gain, bias, log_mix, log_scale):
    x = np.ascontiguousarray(np.asarray(x, dtype=np.float32))
    W_q = np.asarray(W_q, dtype=np.float32)
    W_k = np.asarray(W_k, dtype=np.float32)
    gain = np.asarray(gain, dtype=np.float32)
    bias = np.asarray(bias, dtype=np.float32)
    mix = float(1.0 / (1.0 + math.exp(-float(log_mix))))
    scale = float(np.log1p(np.exp(np.float32(log_scale))) + np.float32(0.01))

    if "prog" not in _prog_cache:
        _prog_cache["prog"] = _build_program()
    nc = _prog_cache["prog"]

    # ---- host-side layout prep ----
    xh = x.astype(np.float16)
    hf = xh.astype(np.float32)
    xl = ((x - hf) * SPLIT).astype(np.float16)
    h32 = (hf * 32.0).astype(np.float16)     # exact power-of-2 scalings
    h64 = (hf * 64.0).astype(np.float16)
    h8 = xh.astype(FP8)
    l8 = xl.astype(FP8)

    ident16 = np.eye(P, dtype=np.float16)
    identbf = np.eye(P, dtype=np.float32).astype(ml_dtypes.bfloat16)
    identf8 = np.eye(P, dtype=np.float32).astype(FP8E5)
    wq = np.ascontiguousarray(
        (W_q / (32.0 * math.sqrt(DH))).T.astype(np.float16)
        .reshape(NCHUNK, P, DH).transpose(1, 0, 2))
    wk = np.ascontiguousarray(
        (W_k / 64.0).T.astype(np.float16)
        .reshape(NCHUNK, P, DH).transpose(1, 0, 2))
    gainb = ((1.0 - mix) * gain).astype(np.float32)   # folded into xbh

    per_batch = {}
    for b in range(B):
        pan16 = np.ascontiguousarray(
            h64[b].T.reshape(NCHUNK, P, NPAN, PANEL).transpose(2, 1, 0, 3))
        l8T = l8[b].T.reshape(NCHUNK, P, NPAN, PANEL)
        h8T = h8[b].T.reshape(NCHUNK, P, NPAN, PANEL)
        pan8 = np.ascontiguousarray(
            np.stack([l8T, h8T], axis=2).transpose(3, 1, 0, 2, 4))
        xbh = np.ascontiguousarray(
            (x[b] * gainb).astype(np.float16)
            .reshape(NPAN, 4, P, D).transpose(0, 2, 1, 3))
        per_batch[b] = {"pan16": pan16, "pan8": pan8, "xbh": xbh}

    in_maps = []
    for core in range(NCORES):
        b, j = core // 4, core % 4
        rows = np.concatenate(
            [np.arange(P * (4 * l + j), P * (4 * l + j) + P) for l in range(NLEV)])
        xr16 = np.ascontiguousarray(
            h32[b][rows].reshape(NLEV, P, NCHUNK, P).transpose(0, 3, 2, 1))
        h8r = h8[b][rows].reshape(NLEV, P, NCHUNK, P)
        l8r = l8[b][rows].reshape(NLEV, P, NCHUNK, P)
        xr8 = np.ascontiguousarray(
            np.stack([h8r, l8r], axis=3).transpose(0, 4, 2, 3, 1))
        md = np.zeros((P, PANEL), dtype=np.float32)
        k_idx = np.arange(P)[:, None]
        s_idx = np.arange(PANEL)[None, :]
        md[s_idx >= k_idx + P * j] = MASKVAL
        in_maps.append({
            **per_batch[b],
            "xr16": xr16, "xr8": xr8,
            "maskdiag": md.astype(ml_dtypes.bfloat16),
            "wq": wq, "wk": wk,
            "ident16": ident16, "identbf": identbf, "identf8": identf8,
            "eshift": np.full((P, 1), ESHIFT, dtype=np.float32),
        })

    res = run_bass_kernel_spmd(nc, in_maps, core_ids=list(range(NCORES)))
    _prog_cache["last_results"] = res

    # ---- host: assemble msg, blend + gelu + scale, fix flagged rows ----
    msgterm = np.empty((B, T, D), dtype=np.float32)
    flagged = []
    for core in range(NCORES):
        b, j = core // 4, core % 4
        o = res.results[core]["out"]                 # [lev, t, d] fp16
        cnt = res.results[core]["cnt"]               # [lev, t, 1] fp32
        for l in range(NLEV):
            r0 = P * (4 * l + j)
            msgterm[b, r0:r0 + P, :] = o[l].astype(np.float32)
            for r in np.nonzero(cnt[l, :, 0] > 8.25)[0]:
                flagged.append((b, r0 + int(r)))
    _prog_cache["last_flagged"] = len(flagged)

    pre = ((mix * gain) * x + bias + msgterm).astype(np.float64)
    out = (pre * 0.5 * (1.0 + _erf_vec(pre / math.sqrt(2.0)))
           * scale).astype(np.float32)

    # near-tie rows: recompute exactly on host (selection ambiguous on device)
    for b, t in flagged:
        _fix_row(out[b], x[b], W_q, W_k, gain, bias, mix, scale, t)

    # degenerate t=0 rows: uniform attention over ALL positions
    for b in range(B):
        msg0 = x[b].sum(axis=0, dtype=np.float32) * np.float32(1.0 / T)
        blended = np.float32(mix) * x[b, 0] + np.float32(1.0 - mix) * msg0
        pre0 = blended * gain + bias
        out[b, 0, :] = (_gelu_exact_np(pre0.astype(np.float64))
                        * scale).astype(np.float32)
    return out
